# revision 63
# baseline (speedup 1.0000x reference)
"""GAT (2-layer, PyG-style) on 8 Trainium2 NeuronCores.

Math
----
Layer 1 has in_channels=1, so h = x @ W1 is rank-1: every per-edge quantity
reduces to scalars per node.  With s1[h] = sum_c W1[h,c]*att_src1[h,c] and
d1[h] = sum_c W1[h,c]*att_dst1[h,c]:

    e[i,h]   = leaky_relu(s1[h]*x[src_i] + d1[h]*x[dst_i])
    denom[d,h] = sum_{i->d} exp(e[i,h])          (max-shift skipped: |e| < ~10)
    z[d,h]     = sum_{i->d} exp(e[i,h]) * x[src_i]
    out1[d,h,c] = elu(W1[h,c]*z/(denom+eps) + b1[h,c])

Layer 2 (heads=1, out=1) similarly only needs the scalar h2 = out1 @ W2.

Sharding: dst-owner node sharding (12500 nodes/core).  Per core, nodes are
degree-sorted and packed into 98 blocks of 128 (partition dim); each node's
incoming edges occupy W columns (block-group-padded).  x[dst] is then a free
per-partition broadcast and segment sums are row reductions; only x[src]
needs a real gather.

Gather (the whole ballgame)
---------------------------
Per-element indirect DMA costs ~360ns/descriptor (SWDGE desc-gen bound) --
80+ms for 3.4M edge-gathers.  Instead we use the MoE `dma_gather` custom
instruction (mlp ucode library; 1024 idxs per call, larger calls crash).
The table is viewed as [1568, 64] f32 chunks (256B rows, the instruction
minimum); each edge slot fetches the 64-value chunk CONTAINING its source
scalar (idx = pos//64, placed so idx position c*128+p lands at grid slot
(p, c)).  The wanted scalar is then selected by an on-device one-hot mask,
is_equal(off[p,c], iota64) with off=pos%64 (255 for padding slots), times
a free-axis add-reduce.  The gather is SWDGE-queue limited at
max(bytes/22.5B/ns, ~12ns)/descriptor/queue: at 256B elems a 205k-slot
pass (C=1600, ~2.4% slot padding) takes 0.72ms; the default E16 mode
fetches 64B elems (elem_size=16 f32, elem_step=64 -- the %256 assert is
transpose-only; a raw constructor relaxes it) from a sparse table
[4*1568, 64] whose rows hold each 16-value group at a 256B-aligned
address, landing at the ~12ns/desc rate floor: ~0.65ms/pass (paired A/B
+70us/pass vs dense) with 4x lighter mask-selects.  Either mode needs
>= ~8 calls in flight (gsel pool bufs=12; bufs=5 -> 1.24ms/pass).
Measured via REP_GATHER wall-clock slopes and in-process paired A/Bs;
single-shot wall times are useless here because every device call rides
an ~80ms axon RPC floor.

Both layers share ONE idx/off table: the x table and the h2 table are both
laid out in padded PROCESSING order (proc rank i on core k at table row
k*12544 + i), so pos = k(src)*ROWS + inv[src] works for both gathers.  The
host permutes x for free (x_own doubles as the table slice, staged to an
Internal tensor and AllGathered on device), and h2_own stores straight
into the AllGather slice with no reordering.  The [P,C] column mask is
derived on device from off==255, and off ships as uint8.  Inputs pack
into one flat tensor per dtype (f32/i16/u8) -- total ExternalInput
traffic ~0.67MB/core (5.3MB all-core) vs 24.4MB for the two-table
variant, and AllGathers measure ~free (32 extra added ~0 wall).

Self-loops (rank 0 of every dst segment) are folded into denom/zt directly
from x_own/h2_own, shrinking every block's column width by 1.  Layer-1
normalization + epilogue run fused per block-group so they overlap the
gather-1 tail; layer 2 likewise finishes per group (self-loop terms
precomputed from h2_own during the gather-2 fill, output stored per group
to a [NBLK, P] tensor so each store is one contiguous descriptor).
Device exec ~1.7ms/core: 2 x 0.72ms edge-gather passes
(queue-bandwidth floor) + ~0.3ms prologue/inter-pass/tail vector work
(baseline: 3.5ms).  TimelineSim puts the non-gather serial at ~300us;
the queue cap is hard (ucode MAX_SWDGE_QUEUES=4, 1 queue = 1 engine) and
a paired A/B (gather-only repeat passes vs full) shows the mask-selects
add only ~0.03-0.06ms/pass beyond it, so further gains would need a
different gather primitive.  All regular DMAs share one HWDGE queue:
issue order is param/x-staging loads first, big idx/off copies last
(contiguous full-width copies -- column-slicing them fragments the
descriptors and loses more than the earlier start gains).

(Walrus quirks: >2 sem waits per instruction are split onto injected NoOps;
drains likewise. tile's DMASW lane rotation must match the dma_gather
queue_num rotation, so NUM_SWDGE_GLOBAL_SEMS == NQ and dma_gather is the
only Pool-engine DMA in the program.)
"""
import sys
sys.path.insert(0, "/opt/trn_rl_repo")
import re
import numpy as np
import concourse.bass as bass
import concourse.mybir as mybir
import concourse.tile as tile
from concourse.library_config import mlp
from concourse.library_overlay import lower_extended_insts
from concourse.bass_utils import run_bass_kernel_spmd
from bass_rust import ScopedClock, VectorClock

N = 100000
NCORES = 8
NPC = N // NCORES          # nodes per core
P = 128
NBLK = (NPC + P - 1) // P  # 98
ROWS = NBLK * P            # 12544
NEG_SLOPE = 0.2
EPS = 1e-16
BIG_NEG = -1.0e30
USE_ACT_LRELU = False
CHUNK = 64                 # gather element width (f32) = 256B
GC = 8                     # grid columns per dma_gather call (1024 idxs)
NQ = 4                     # SWDGE queues
NCHT = 1568                # table chunks = 100352/64
SL = ROWS // CHUNK         # 196 chunks per per-core table slice
import os
REP_GATHER = int(os.environ.get("REP_GATHER", "1"))   # timing-slope knob
REP_AG = int(os.environ.get("REP_AG", "1"))           # AllGather slope knob
GSEL_BUFS = int(os.environ.get("GSEL_BUFS", "12"))    # in-flight gather tiles
FUSE_L1 = int(os.environ.get("FUSE_L1", "1"))         # per-group epilogue
GATHER_ONLY_REPS = int(os.environ.get("GATHER_ONLY_REPS", "0"))  # slope knob
REP_LIB = int(os.environ.get("REP_LIB", "1"))         # load_library slope knob
E16 = int(os.environ.get("E16", "1"))                 # 64B-elem gathers

F32 = mybir.dt.float32
BF16 = mybir.dt.bfloat16
I16 = mybir.dt.int16
U8 = mybir.dt.uint8
AT = mybir.AluOpType
AF = mybir.ActivationFunctionType


# ---------------------------------------------------------------------------
# Tile tail-drain workaround: walrus TPB_CTRL codegen rejects a Drain with
# more than two sem waits; emit one NOP-wait per proc first.
def _split_drain_and_barrier(self, tick_clock, wait_clock):
    gc = tick_clock.global_clock
    ticks = [int(x) for x in re.findall(r"\d+", repr(gc))]
    for i, t in enumerate(ticks):
        if t <= 0:
            continue
        sub = VectorClock()
        sub.require_at_least(i, t)
        inst = self.nc.sync.nop()
        wait_clock.add_sem_waits(inst.ins, ScopedClock({None: sub}))
    self.nc.sync.drain()
    self.nc.all_engine_barrier()
    popped = self.nc._tile_sem_poison_stack.pop()
    assert popped is self._sem_poison
    self.nc.clear_and_free_semaphores(list(self.sems.allocated().values()))
    self.nc.all_engine_barrier()


tile.TileContext._drain_and_barrier = _split_drain_and_barrier

import concourse.tile_sem_assignment as _tsa
_tsa.NUM_SWDGE_GLOBAL_SEMS = NQ
_tsa.NUM_HWDGE_SEMS = 1

# Walrus encodes at most ~2 sem waits per instruction; split any excess onto
# injected same-engine NoOps in the BIR JSON right before compilation.
import json as _json
from concourse import bass2jax as _b2j


def _split_waits_json(bir, max_keep=1):
    d = _json.loads(bir)
    ctr = [0]

    def fix_block(blk):
        out = []
        for inst in blk.get("instructions", []):
            si = inst.get("sync_info")
            waits = (si or {}).get("on_wait") or []
            if len(waits) > max_keep and inst.get("opcode") != "NoOp":
                keep = waits[-max_keep:]
                for w in waits[:-max_keep]:
                    ctr[0] += 1
                    out.append({"debug": inst.get("debug", 0),
                                "engine": inst["engine"], "ins": [],
                                "outs": [], "name": f"I-wsp{ctr[0]}",
                                "opcode": "NoOp",
                                "sync_info": {"on_update": [], "on_wait": [w]}})
                si["on_wait"] = keep
            out.append(inst)
        blk["instructions"] = out
        for sb in blk.get("blocks", []):
            fix_block(sb)

    for fn in d["functions"]:
        for blk in fn["blocks"]:
            fix_block(blk)
    return _json.dumps(d).encode()


if not getattr(_b2j, "_wsplit_patched", False):
    _orig_cbk = _b2j.compile_bir_kernel

    def _cbk(bir, *a, **k):
        return _orig_cbk(_split_waits_json(bir), *a, **k)

    _b2j.compile_bir_kernel = _cbk
    _b2j._wsplit_patched = True


# ---------------------------------------------------------------------------
# CPU-side structural prep (graph topology only, no float math)

def _prep(edge_index):
    src = np.asarray(edge_index[0], dtype=np.int64)
    dst = np.asarray(edge_index[1], dtype=np.int64)
    loop = np.arange(N, dtype=np.int64)
    src = np.concatenate([loop, src])
    dst = np.concatenate([loop, dst])

    deg = np.bincount(dst, minlength=N)

    perms = []        # per core: processing order (local node ids 0..NPC-1)
    inv_all = np.zeros((NCORES, NPC), dtype=np.int64)   # orig -> proc rank
    blk_deg = np.zeros((NCORES, NBLK), dtype=np.int64)
    for k in range(NCORES):
        dk = deg[k * NPC:(k + 1) * NPC]
        order = np.argsort(-dk, kind="stable").astype(np.int64)
        perms.append(order)
        inv_all[k][order] = np.arange(NPC)
        dks = np.concatenate([dk[order], np.zeros(ROWS - NPC, np.int64)])
        blk_deg[k] = dks.reshape(NBLK, P).max(1)

    # Uniform group structure across cores: W per block = max over cores,
    # then greedily merge consecutive blocks (pad to group max) keeping the
    # added padding under ~8% and the per-group volume bounded.
    # Self-loops (rank 0 of every node, value available on-core) are folded
    # into the segment sums directly, so the grid only holds ranks >= 1.
    wblk = blk_deg.max(0) - 1      # [NBLK], non-increasing
    groups = []                    # list of (start_blk, end_blk, W)
    g0 = 0
    waste = 0.0
    real = 1.0
    for b in range(1, NBLK + 1):
        merge = False
        if b < NBLK:
            new_waste = waste + (wblk[g0] - wblk[b])
            new_real = real + wblk[b]
            vol = (b + 1 - g0) * max(wblk[g0], 1)
            if new_waste <= 0.02 * new_real and vol <= 128:
                merge = True
        if merge:
            waste, real = new_waste, new_real
            continue
        groups.append((g0, b, int(max(wblk[g0], 1))))
        if b < NBLK:
            g0 = b
            waste = 0.0
            real = float(wblk[b])
    col_off = np.zeros(NBLK, dtype=np.int64)
    C = 0
    for (a, b, w) in groups:
        for blk in range(a, b):
            col_off[blk] = C
            C += w
    if C % GC:
        C += GC - C % GC        # gather-call granularity

    # Both gather tables (x and h2) live in padded PROCESSING order: the
    # node with processing rank i on core k sits at table row k*ROWS + i.
    # One idx/off table serves both layers, and h2_own stores straight
    # into the AllGather slice with no reordering (the host permutes x
    # for free, and unpermutes the output).
    pos = np.zeros((NCORES, P, C), dtype=np.int64)
    valid = np.zeros((NCORES, P, C), dtype=bool)

    order = np.argsort(dst, kind="stable")
    src_s = src[order]
    dst_s = dst[order]
    starts = np.searchsorted(dst_s, np.arange(N + 1))

    for k in range(NCORES):
        inv_k = inv_all[k]
        base = k * NPC
        lo, hi = starts[base], starts[base + NPC]
        d_loc = dst_s[lo:hi] - base                  # local dst id
        i_proc = inv_k[d_loc]                        # processing index
        blk = i_proc // P
        p = i_proc - blk * P
        seg_start = starts[d_loc + base] - lo        # rank within segment
        rank = np.arange(hi - lo) - seg_start
        nsl = rank >= 1                              # rank 0 = self-loop
        c = col_off[blk[nsl]] + rank[nsl] - 1
        p = p[nsl]
        s = src_s[lo:hi][nsl]
        ks = s // NPC
        pos[k, p, c] = ks * ROWS + inv_all[ks, s % NPC]
        valid[k, p, c] = True

    return {
        "groups": groups, "C": C, "perms": perms,
        "pos": pos, "valid": valid,
    }


def _mk_gather_inputs(pos, valid, C):
    """idx tile [16, ncalls*64] int16 + within-chunk offsets [P, C] uint8
    (invalid slots get 255 so the on-device iota-eq mask comes out 0)."""
    ncalls = C // GC
    ce = 16 if E16 else CHUNK
    chunk = (pos // ce).astype(np.int16)             # [P, C]
    off = np.where(valid, pos % ce, 255).astype(np.uint8)
    arr = chunk.reshape(P, ncalls, GC).transpose(1, 2, 0).reshape(
        ncalls, GC * P)                              # [g, j] j=c_local*128+p
    w = arr.reshape(ncalls, (GC * P) // 16, 16).transpose(0, 2, 1)
    t16 = np.ascontiguousarray(
        w.transpose(1, 0, 2).reshape(16, ncalls * 64).astype(np.int16))
    return t16, np.ascontiguousarray(off)


# ---------------------------------------------------------------------------
# Bass program (identical for all cores; per-core data differs)

def _build(C, groups):
    nc = bass.Bass("TRN2", target_bir_lowering=False, debug=False,
                   num_devices=NCORES, num_swdge_queues=NQ,
                   dynamic_dma_scratch_size=65536)
    ncalls = C // GC
    # Inputs ship as ONE tensor per dtype (fewer host->device transfers);
    # the named tensors below are flat-offset views into the blobs.
    # f32 layout: x_own | iota64 | w1 | as1 | ad1 | b1 | w2 | sc2
    FTOT = P * NBLK + 64 + 5 * 128 + 8
    ITOT = 16 * ncalls * 64
    UTOT = P * C
    fb = nc.dram_tensor("fblob", [FTOT], F32, kind="ExternalInput").ap()
    ib = nc.dram_tensor("iblob", [ITOT], I16, kind="ExternalInput").ap()
    ub = nc.dram_tensor("ublob", [UTOT], U8, kind="ExternalInput").ap()

    def fview(_lo, _hi, pat, **kw):
        return fb[_lo:_hi].rearrange(pat, **kw)

    o = P * NBLK
    x_own = fview(0, o, "(p b) -> p b", p=P)
    iota64 = fview(o, o + 64, "(a c) -> a c", a=1); o += 64
    w1 = fview(o, o + 128, "(a c) -> a c", a=1); o += 128
    as1 = fview(o, o + 128, "(a c) -> a c", a=1); o += 128
    ad1 = fview(o, o + 128, "(a c) -> a c", a=1); o += 128
    b1 = fview(o, o + 128, "(a c) -> a c", a=1); o += 128
    w2 = fview(o, o + 128, "(a c) -> a c", a=1); o += 128
    sc2 = fview(o, o + 8, "(a c) -> a c", a=1); o += 8
    assert o == FTOT
    # sc2 row: [att_src2, att_dst2, b2, 0 | iota4]
    idxa = ib[0:ITOT].rearrange("(r c) -> r c", r=16)
    offa = ub[0:UTOT].rearrange("(p c) -> p c", p=P)

    out_d = nc.dram_tensor("out", [NBLK, P], F32, kind="ExternalOutput").ap()
    x_int = nc.dram_tensor("x_int", [SL, CHUNK], F32, kind="Internal").ap()
    xt_full = nc.dram_tensor("xt_full", [NCHT, CHUNK], F32,
                             kind="Internal", addr_space="Shared").ap()
    h2sl = nc.dram_tensor("h2sl", [SL, CHUNK], F32, kind="Internal").ap()
    h2t_full = nc.dram_tensor("h2t_full", [NCHT, CHUNK], F32,
                              kind="Internal", addr_space="Shared").ap()
    xt16 = nc.dram_tensor("xt16", [NCHT * 4, CHUNK], F32,
                          kind="Internal", addr_space="Shared").ap()
    x_i16 = nc.dram_tensor("x_i16", [SL * 4, CHUNK], F32,
                           kind="Internal").ap()
    h2t16 = nc.dram_tensor("h2t16", [NCHT * 4, CHUNK], F32,
                           kind="Internal").ap()

    with tile.TileContext(nc, num_cores=NCORES) as tc:
        _body(nc, tc, C, groups, x_own, idxa, offa, iota64,
              w1, as1, ad1, b1, w2, sc2, out_d,
              x_int, xt_full, h2sl, h2t_full, xt16, h2t16, x_i16)
    lower_extended_insts(nc)
    return nc


def _dma_gather_raw(g, out_ap, in_ap, idxs_ap, num_idxs, num_idxs_reg,
                    elem_size, elem_step, queue_num):
    """dma_gather with the elem%256B assert relaxed to %64B (the 256B rule
    is annotated 'transpose restriction' in bass; stride still must be a
    256B multiple, which elem_step=64 f32 satisfies)."""
    g._assert_queue_num(queue_num)
    assert idxs_ap.dtype == mybir.dt.int16
    elem_size_bytes = elem_size * mybir.dt.size(in_ap.dtype)
    assert elem_size_bytes % 64 == 0
    stride_bytes = elem_step * mybir.dt.size(in_ap.dtype)
    assert stride_bytes % 256 == 0
    stride_bytes_256 = stride_bytes // 256
    assert stride_bytes_256 < 256
    assert in_ap.ap[-1][1] == out_ap.ap[-1][1] == elem_size
    assert in_ap.ap[0][0] == elem_step
    _in_ap = g.lower_ap_dma(in_ap, for_custom_bir_dma=True)
    _idxs_ap = g.lower_ap(idxs_ap)
    _out_ap = g.lower_ap(out_ap)
    return g.add_instruction(
        mybir.InstDMAGatherAnt(
            name=g.bass.get_next_instruction_name(),
            ins=[*_in_ap, _idxs_ap,
                 g.lower_val_access(g.to_reg(num_idxs_reg))],
            outs=[_out_ap],
            transpose=False, num_idxs=num_idxs, elem_size=elem_size,
            stride_bytes_256=stride_bytes_256, gen_mode=0,
            single_packet=True, queue_num=queue_num,
            sbuf_tokens_per_rank=0, sbuf_free_dim_per_rank=0,
            sbuf_free_dim_pad_per_rank=0, sbuf_byte_offset=0))


def _sparsify16(nc, const, dense, sparse, tag):
    """dense [NCHT,64] -> sparse [NCHT*4,64] rows r=pos//16 start at
    256B-aligned addresses with the 16 payload values in cols 0:16."""
    t = const.tile([P, NCHT * CHUNK // P], F32, tag=tag)
    nc.sync.dma_start(
        t[:], dense[:].rearrange("r c -> (r c)").rearrange(
            "(p f) -> p f", p=P))
    nc.sync.dma_start(
        sparse[:].rearrange("(p g) c -> p g c", p=P)[:, :, 0:16],
        t[:].rearrange("p (g pl) -> p g pl", pl=16))


def _gather_v2(nc, tc, C, table_d, idx_t, off_t, iota_t, xs, nreg, qctr,
               do_select=True):
    """xs[p, c] = table.flat[pos[p, c]] via chunked dma_gather + mask-select.

    Per call: 1024 chunk-indices -> [128, GC, 64] f32; the one-hot select
    mask is built on-device as is_equal(off[p,c], iota[k]) (invalid slots
    carry off=255 so they select nothing), then mask-mult + free-axis add
    reduce pick out the wanted scalar per slot."""
    ncalls = C // GC
    E = 16 if E16 else CHUNK
    with tc.tile_pool(name="gsel", bufs=GSEL_BUFS) as gp:
        for g in range(ncalls):
            ch = gp.tile([P, GC * E], F32, tag="ch")
            ch3 = ch[:].rearrange("p (g k) -> p g k", k=E)
            if E16:
                _dma_gather_raw(
                    nc.gpsimd, ch3, table_d[:, 0:16],
                    idx_t[:, g * 64:(g + 1) * 64],
                    GC * P, nreg, 16, CHUNK, qctr[0] % NQ)
            else:
                nc.gpsimd.dma_gather(
                    ch3, table_d, idx_t[:, g * 64:(g + 1) * 64],
                    GC * P, nreg, CHUNK, queue_num=qctr[0] % NQ)
            qctr[0] += 1
            if not do_select:
                continue
            m = gp.tile([P, GC * E], F32, tag="m")
            nc.vector.tensor_tensor(
                out=m[:].rearrange("p (g k) -> p g k", k=E),
                in0=off_t[:, g * GC:(g + 1) * GC].rearrange(
                    "p g -> p g ()").to_broadcast([P, GC, E]),
                in1=iota_t[:, 0:E].rearrange("p k -> p () k").to_broadcast(
                    [P, GC, E]),
                op=AT.is_equal)
            nc.vector.tensor_tensor(out=ch[:], in0=ch[:], in1=m[:],
                                    op=AT.mult)
            nc.vector.tensor_reduce(
                out=xs[:, g * GC:(g + 1) * GC],
                in_=ch3, axis=mybir.AxisListType.X, op=AT.add)


def _body(nc, tc, C, groups, x_own_d, idxa_d, offa_d,
          iota64_d, w1_d, as1_d, ad1_d, b1_d, w2_d, sc2_d, out_d,
          x_int, xt_full, h2sl, h2t_full, xt16, h2t16, x_i16):
    import contextlib
    ctx = contextlib.ExitStack()
    H = 8
    ncalls = C // GC
    with ctx:
        for _ in range(REP_LIB):
            nc.gpsimd.load_library(mlp)
        nreg = nc.gpsimd.to_reg(GC * P)
        qctr = [0]
        const = ctx.enter_context(tc.tile_pool(name="const", bufs=1))
        group_c0 = {}
        _c = 0
        for (ga, gb, gw) in groups:
            group_c0[ga] = _c
            _c += gw * (gb - ga)

        # ---- persistent loads.  Issue order matters: all regular DMAs
        # share ONE HWDGE queue (NUM_HWDGE_SEMS=1), so the small param
        # loads and the x staging chain (which gates the AllGather, which
        # gates the first gather call) must queue BEFORE the ~27us of
        # idx-broadcast copies.
        x_own = const.tile([P, NBLK], F32)
        nc.sync.dma_start(x_own[:], x_own_d[:])
        if E16:
            # proc idx i = b2*128 + ph*16 + pl -> sparse row b2*8+ph, col pl
            xv = x_i16[:].rearrange("(b2 ph) c -> b2 ph c", ph=8)
            for ph in range(8):
                nc.sync.dma_start(
                    xv[:, ph:ph + 1, 0:16].rearrange(
                        "b2 a pl -> (a pl) b2"),
                    x_own[ph * 16:(ph + 1) * 16, :])
            for _ in range(REP_AG):
                nc.gpsimd.collective_compute(
                    "AllGather", AT.bypass,
                    replica_groups=[list(range(NCORES))],
                    ins=[x_i16[:]], outs=[xt16[:]])
        else:
            nc.sync.dma_start(
                x_int[:].rearrange("r c -> (r c)").rearrange(
                    "(b p) -> p b", p=P),
                x_own[:])
            for _ in range(REP_AG):
                nc.gpsimd.collective_compute(
                    "AllGather", AT.bypass,
                    replica_groups=[list(range(NCORES))],
                    ins=[x_int[:]], outs=[xt_full[:]])

        # ---- params: one row, then broadcast via ones-matmul
        # 0:128 w1 | 128:256 as1 | 256:384 ad1 | 384:512 b1 | 512:640 w2
        # 640:648 sc2 (att_src2, att_dst2, b2, w2sum | iota4)
        # 648:656 s1 | 656:664 d1 | 664:728 iota64
        prow = const.tile([1, 728], F32)
        nc.sync.dma_start(prow[:, 0:128], w1_d[:])
        nc.sync.dma_start(prow[:, 128:256], as1_d[:])
        nc.sync.dma_start(prow[:, 256:384], ad1_d[:])
        nc.sync.dma_start(prow[:, 384:512], b1_d[:])
        nc.sync.dma_start(prow[:, 512:640], w2_d[:])
        nc.sync.dma_start(prow[:, 640:648], sc2_d[:])
        nc.sync.dma_start(prow[:, 664:728], iota64_d[:])
        tmp = const.tile([1, 256], F32)
        nc.vector.tensor_tensor(out=tmp[:, 0:128], in0=prow[:, 0:128],
                                in1=prow[:, 128:256], op=AT.mult)
        nc.vector.tensor_tensor(out=tmp[:, 128:256], in0=prow[:, 0:128],
                                in1=prow[:, 256:384], op=AT.mult)
        nc.vector.tensor_reduce(out=prow[:, 648:664],
                                in_=tmp[:].rearrange("a (h c) -> a h c", c=16),
                                axis=mybir.AxisListType.X, op=AT.add)
        nc.vector.tensor_reduce(out=prow[:, 643:644], in_=prow[:, 512:640],
                                axis=mybir.AxisListType.X, op=AT.add)

        ones = const.tile([1, P], F32)
        nc.vector.memset(ones[:], 1.0)
        # funnel prow through one DVE copy so the matmul (whose load-weights
        # encoding has a tight sem-wait budget) depends on a single producer
        prow2 = const.tile([1, 728], F32)
        nc.vector.tensor_copy(out=prow2[:], in_=prow[:])
        psum = ctx.enter_context(tc.tile_pool(name="psum", bufs=2,
                                              space="PSUM"))
        pc = const.tile([P, 728], F32)
        for lo, hi in ((0, 512), (512, 728)):
            pcast = psum.tile([P, 512], F32, tag="pcast")
            nc.tensor.matmul(pcast[:, :hi - lo], lhsT=ones[:],
                             rhs=prow2[:, lo:hi], start=True, stop=True)
            nc.vector.tensor_copy(out=pc[:, lo:hi], in_=pcast[:, :hi - lo])
        W1t = pc[:, 0:128]
        B1t = pc[:, 384:512]
        W2t = pc[:, 512:640]
        s2c = pc[:, 640:641]
        d2c = pc[:, 641:642]
        b2c = pc[:, 642:643]
        w2sum = pc[:, 643:644]
        s1c = pc[:, 648:656]
        d1c = pc[:, 656:664]
        iota_t = pc[:, 664:728]

        # ---- idx/off tables (big contiguous copies; queued after
        # everything that gates the pipeline head)
        idx_t = const.tile([P, ncalls * 64], I16)
        for k in range(8):
            nc.sync.dma_start(idx_t[16 * k:16 * (k + 1), :], idxa_d[:])
        off8 = const.tile([P, C], U8)
        nc.sync.dma_start(off8[:], offa_d[:])
        off_t = const.tile([P, C], F32)
        nc.vector.tensor_copy(out=off_t[:], in_=off8[:])

        # mask: BIG_NEG where the slot is padding (off == 255)
        bigneg = const.tile([P, 1], F32)
        nc.vector.memset(bigneg[:], BIG_NEG)
        mneg = const.tile([P, C], F32)
        nc.vector.scalar_tensor_tensor(
            out=mneg[:], in0=off_t[:], scalar=255.0,
            in1=bigneg[:].to_broadcast([P, C]),
            op0=AT.is_equal, op1=AT.mult)

        # ---- gather x[src] (layer 1; xt16 AllGathered sparse directly)
        xs = const.tile([P, C], F32)
        if REP_GATHER == 0:          # timing knob: no-gather variant
            nc.vector.memset(xs[:], 0.5)
        for rep in range(REP_GATHER):
            _gather_v2(nc, tc, C, xt16 if E16 else xt_full, idx_t, off_t,
                       iota_t, xs,
                       nreg, qctr, do_select=(rep == REP_GATHER - 1
                                              or not GATHER_ONLY_REPS))

        # adst[p, b, h] = x_own[p, b] * d1[h]
        adst = const.tile([P, NBLK * H], F32)
        nc.vector.tensor_tensor(
            out=adst[:].rearrange("p (b h) -> p b h", h=H),
            in0=x_own[:].rearrange("p b -> p b ()").to_broadcast([P, NBLK, H]),
            in1=d1c.rearrange("p h -> p () h").to_broadcast([P, NBLK, H]),
            op=AT.mult)

        denom = const.tile([P, NBLK * H], F32)
        zt = const.tile([P, NBLK * H], F32)
        sl1 = const.tile([P, H], F32)
        nc.vector.tensor_tensor(out=sl1[:], in0=s1c, in1=d1c, op=AT.add)
        r = const.tile([P, NBLK * H], F32)
        h2_own = const.tile([P, NBLK], F32)

        # ---- layer 1, fused per block-group so the self-loop fold, the
        # alpha normalization, and the epilogue for group g all run while
        # later groups are still gathering.
        with tc.tile_pool(name="work", bufs=2) as work, \
                tc.tile_pool(name="ep", bufs=2) as ep:
            h2fl = h2sl[:].rearrange("r c -> (r c)")
            for (a, b, w) in groups:
                _layer1_main(nc, C, [(a, b, w)], group_c0, work, xs, mneg,
                             adst, s1c, denom, zt)
                if FUSE_L1:
                    _fold_norm_group(nc, work, a, b, x_own, sl1, denom, zt,
                                     r)
                    _epilogue(nc, ep, r, W1t, B1t, W2t, h2_own, w2sum, a, b)
                    # store this group's h2 slice (flat b*128+p = proc
                    # index, one contiguous descriptor per group)
                    nc.sync.dma_start(
                        h2fl[a * P:b * P].rearrange(
                            "(b2 p) -> p b2", p=P),
                        h2_own[:, a:b])
            if not FUSE_L1:
                _fold_norm_group(nc, work, 0, NBLK, x_own, sl1, denom, zt, r)
                _epilogue(nc, ep, r, W1t, B1t, W2t, h2_own, w2sum, 0, NBLK)
                nc.sync.dma_start(
                    h2fl.rearrange("(b p) -> p b", p=P), h2_own[:])
        _rest(nc, tc, C, groups, group_c0, const, mneg, h2_own, idx_t,
              off_t, iota_t, s2c, d2c, b2c, out_d,
              h2sl, h2t_full, h2t16, nreg, qctr)


def _layer1_main(nc, C, groups, group_c0, work, xs, mneg, adst, s1c,
                 denom, zt):
        H = 8
        for (a, b, w) in groups:
            nb = b - a
            c0 = group_c0[a]
            V = nb * H * w
            xs_g = xs[:, c0:c0 + nb * w].rearrange("p (n w) -> p n () w", w=w)
            mn_g = mneg[:, c0:c0 + nb * w].rearrange("p (n w) -> p n () w", w=w)
            ad_g = adst[:, a * H:b * H].rearrange("p (n h) -> p n h ()", h=H)
            s1_g = s1c.rearrange("p h -> p () h ()")

            u = work.tile([P, V], F32, tag="u")
            u4 = u[:].rearrange("p (n h w) -> p n h w", h=H, w=w)
            nc.vector.tensor_tensor(out=u4, in0=xs_g.to_broadcast([P, nb, H, w]),
                                    in1=s1_g.to_broadcast([P, nb, H, w]), op=AT.mult)
            u2 = work.tile([P, V], F32, tag="u2")
            u24 = u2[:].rearrange("p (n h w) -> p n h w", h=H, w=w)
            nc.vector.tensor_tensor(out=u24, in0=u4,
                                    in1=ad_g.to_broadcast([P, nb, H, w]), op=AT.add)
            nc.vector.tensor_tensor(out=u4, in0=u24,
                                    in1=mn_g.to_broadcast([P, nb, H, w]), op=AT.add)
            # leaky relu: max(0.2*v, v), then exp
            if USE_ACT_LRELU:
                nc.scalar.activation(out=u24, in_=u4, func=AF.Lrelu,
                                     alpha=NEG_SLOPE)
            else:
                nc.vector.scalar_tensor_tensor(out=u24, in0=u4, scalar=NEG_SLOPE,
                                               in1=u4, op0=AT.mult, op1=AT.max)
            ex = work.tile([P, V], F32, tag="ex")
            ex4 = ex[:].rearrange("p (n h w) -> p n h w", h=H, w=w)
            nc.scalar.activation(out=ex4, in_=u24, func=AF.Exp)
            nc.vector.tensor_reduce(
                out=denom[:, a * H:b * H].rearrange("p (n h) -> p n h", h=H),
                in_=ex4, axis=mybir.AxisListType.X, op=AT.add)
            nc.vector.tensor_tensor(out=u4, in0=ex4,
                                    in1=xs_g.to_broadcast([P, nb, H, w]), op=AT.mult)
            nc.vector.tensor_reduce(
                out=zt[:, a * H:b * H].rearrange("p (n h) -> p n h", h=H),
                in_=u4, axis=mybir.AxisListType.X, op=AT.add)

def _fold_norm_group(nc, work, a, b, x_own, sl1, denom, zt, r):
        """Blocks [a, b): fold the self-loop edge (src == dst) into denom/zt
        -- e_self = lrelu((s1+d1)*x_own); denom += exp(e_self); zt +=
        exp(e_self)*x_own (pad rows add exp(0)=1 to dead nodes) -- then
        r = zt / (denom + eps)."""
        H = 8
        nb = b - a
        xo_g = x_own[:, a:b].rearrange("p b -> p b ()").to_broadcast(
            [P, nb, H])
        us = work.tile([P, nb * H], F32, tag="u")
        usv = us[:, :nb * H]
        us3 = usv.rearrange("p (b h) -> p b h", h=H)
        nc.vector.tensor_tensor(
            out=us3, in0=xo_g,
            in1=sl1[:].rearrange("p h -> p () h").to_broadcast([P, nb, H]),
            op=AT.mult)
        nc.vector.scalar_tensor_tensor(out=usv, in0=usv, scalar=NEG_SLOPE,
                                       in1=usv, op0=AT.mult, op1=AT.max)
        us2 = work.tile([P, nb * H], F32, tag="u2")
        us2v = us2[:, :nb * H]
        nc.scalar.activation(out=us2v, in_=usv, func=AF.Exp)
        dn = denom[:, a * H:b * H]
        zv = zt[:, a * H:b * H]
        nc.vector.tensor_tensor(out=dn, in0=dn, in1=us2v, op=AT.add)
        nc.vector.tensor_tensor(
            out=us2v.rearrange("p (b h) -> p b h", h=H),
            in0=us2v.rearrange("p (b h) -> p b h", h=H),
            in1=xo_g, op=AT.mult)
        nc.vector.tensor_tensor(out=zv, in0=zv, in1=us2v, op=AT.add)
        rv = r[:, a * H:b * H]
        nc.vector.tensor_scalar(out=rv, in0=dn, scalar1=float(EPS),
                                scalar2=None, op0=AT.add)
        nc.vector.reciprocal(out=rv, in_=rv)
        nc.vector.tensor_tensor(out=rv, in0=rv, in1=zv, op=AT.mult)


def _epilogue(nc, ep, r, W1t, B1t, W2t, h2_own, w2sum, ga, gb):
        H = 8
        EPB = 7
        for a in range(ga, gb, EPB):
            b = min(a + EPB, gb)
            nb = b - a
            V = nb * 128
            v = ep.tile([P, EPB * 128], F32, tag="v")
            v4 = v[:, :V].rearrange("p (n h c) -> p n h c", h=H, c=16)
            r_g = r[:, a * H:b * H].rearrange("p (n h) -> p n h ()", h=H)
            w1_g = W1t.rearrange("p (h c) -> p () h c", c=16)
            b1_g = B1t.rearrange("p (h c) -> p () h c", c=16)
            nc.vector.tensor_tensor(out=v4, in0=r_g.to_broadcast([P, nb, H, 16]),
                                    in1=w1_g.to_broadcast([P, nb, H, 16]),
                                    op=AT.mult)
            v2 = ep.tile([P, EPB * 128], F32, tag="v2")
            nc.vector.tensor_tensor(
                out=v2[:, :V].rearrange("p (n h c) -> p n h c", h=H, c=16),
                in0=v4, in1=b1_g.to_broadcast([P, nb, H, 16]), op=AT.add)
            # h1' = max(v,0) + min(exp(v),1);  elu(v) = h1' - 1
            ev = ep.tile([P, EPB * 128], F32, tag="ev")
            nc.scalar.activation(out=ev[:, :V], in_=v2[:, :V], func=AF.Exp)
            nc.vector.tensor_scalar(out=ev[:, :V], in0=ev[:, :V], scalar1=1.0,
                                    scalar2=None, op0=AT.min)
            nc.vector.tensor_scalar(out=v2[:, :V], in0=v2[:, :V], scalar1=0.0,
                                    scalar2=None, op0=AT.max)
            nc.vector.tensor_tensor(out=v2[:, :V], in0=v2[:, :V], in1=ev[:, :V],
                                    op=AT.add)
            # h2 = sum h1'*W2 - W2sum  (the elu -1 folded into W2sum)
            w2_g = W2t.rearrange("p (h c) -> p () (h c)", c=16)
            nc.vector.tensor_tensor(
                out=v4, in0=v2[:, :V].rearrange("p (n f) -> p n f", f=128),
                in1=w2_g.to_broadcast([P, nb, 128]), op=AT.mult)
            nc.vector.tensor_reduce(
                out=h2_own[:, a:b], in_=v4.rearrange("p n h c -> p n (h c)"),
                axis=mybir.AxisListType.X, op=AT.add)
            nc.vector.tensor_scalar(out=h2_own[:, a:b], in0=h2_own[:, a:b],
                                    scalar1=w2sum, scalar2=None,
                                    op0=AT.subtract)
def _rest(nc, tc, C, groups, group_c0, const, mneg, h2_own, idx_t,
          off_t, iota_t, s2c, d2c, b2c, out_d,
          h2sl, h2t_full, h2t16, nreg, qctr):
        # ---- own h2 slice already stored per group; AllGather the table.
        nc.gpsimd.collective_compute(
            "AllGather", AT.bypass,
            replica_groups=[list(range(NCORES))],
            ins=[h2sl[:]], outs=[h2t_full[:]])

        if E16:
            _sparsify16(nc, const, h2t_full, h2t16, "sp16b")
        # ---- layer 2: same idx/off tables as layer 1 (same table layout)
        h2s = const.tile([P, C], F32)
        if REP_GATHER == 0:          # timing knob: no-gather variant
            nc.vector.memset(h2s[:], 0.5)
        for rep in range(REP_GATHER):
            _gather_v2(nc, tc, C, h2t16 if E16 else h2t_full, idx_t, off_t,
                       iota_t, h2s,
                       nreg, qctr, do_select=(rep == REP_GATHER - 1
                                              or not GATHER_ONLY_REPS))

        adst2 = const.tile([P, NBLK], F32)
        nc.vector.tensor_scalar(out=adst2[:], in0=h2_own[:], scalar1=d2c,
                                scalar2=None, op0=AT.mult)

        # ---- layer-2 self-loop terms depend only on h2_own: compute the
        # full tiles up front (they hide under the gather-2 pipeline) so
        # each group can finish and store its output slice immediately.
        sl2 = const.tile([P, 1], F32)
        nc.vector.tensor_tensor(out=sl2[:], in0=s2c, in1=d2c, op=AT.add)
        u2s = const.tile([P, NBLK], F32)
        nc.vector.tensor_scalar(out=u2s[:], in0=h2_own[:], scalar1=sl2,
                                scalar2=None, op0=AT.mult)
        nc.vector.scalar_tensor_tensor(out=u2s[:], in0=u2s[:],
                                       scalar=NEG_SLOPE, in1=u2s[:],
                                       op0=AT.mult, op1=AT.max)
        u2e = const.tile([P, NBLK], F32)
        nc.scalar.activation(out=u2e[:], in_=u2s[:], func=AF.Exp)
        u2h = const.tile([P, NBLK], F32)
        nc.vector.tensor_tensor(out=u2h[:], in0=u2e[:], in1=h2_own[:],
                                op=AT.mult)

        den2 = const.tile([P, NBLK], F32)
        z2 = const.tile([P, NBLK], F32)
        with tc.tile_pool(name="work2", bufs=2) as work:
            _layer2_main(nc, groups, group_c0, work, h2s, mneg, adst2, s2c,
                         den2, z2, u2e, u2h, b2c, out_d)


def _layer2_main(nc, groups, group_c0, work, h2s, mneg, adst2, s2c, den2, z2,
                 u2e, u2h, b2c, out_d):
        for (a, b, w) in groups:
            nb = b - a
            c0 = group_c0[a]
            V = nb * w
            sl = slice(c0, c0 + V)
            h2s_g = h2s[:, sl].rearrange("p (n w) -> p n w", w=w)
            u = work.tile([P, V], F32, tag="u")
            u3 = u[:].rearrange("p (n w) -> p n w", w=w)
            nc.vector.scalar_tensor_tensor(
                out=u3, in0=h2s_g, scalar=s2c,
                in1=adst2[:, a:b].rearrange("p n -> p n ()").to_broadcast(
                    [P, nb, w]),
                op0=AT.mult, op1=AT.add)
            u2 = work.tile([P, V], F32, tag="u2")
            u23 = u2[:].rearrange("p (n w) -> p n w", w=w)
            nc.vector.tensor_tensor(
                out=u23, in0=u3,
                in1=mneg[:, sl].rearrange("p (n w) -> p n w", w=w), op=AT.add)
            if USE_ACT_LRELU:
                nc.scalar.activation(out=u3, in_=u23, func=AF.Lrelu,
                                     alpha=NEG_SLOPE)
            else:
                nc.vector.scalar_tensor_tensor(out=u3, in0=u23, scalar=NEG_SLOPE,
                                               in1=u23, op0=AT.mult, op1=AT.max)
            nc.scalar.activation(out=u23, in_=u3, func=AF.Exp)
            dn = den2[:, a:b]
            nc.vector.tensor_reduce(out=dn, in_=u23,
                                    axis=mybir.AxisListType.X, op=AT.add)
            nc.vector.tensor_tensor(out=u3, in0=u23, in1=h2s_g, op=AT.mult)
            zv = z2[:, a:b]
            nc.vector.tensor_reduce(out=zv, in_=u3,
                                    axis=mybir.AxisListType.X, op=AT.add)
            # fold self-loop, normalize, bias, store this group's output
            nc.vector.tensor_tensor(out=dn, in0=dn, in1=u2e[:, a:b],
                                    op=AT.add)
            nc.vector.tensor_scalar(out=dn, in0=dn, scalar1=float(EPS),
                                    scalar2=None, op0=AT.add)
            nc.vector.reciprocal(out=dn, in_=dn)
            nc.vector.tensor_tensor(out=zv, in0=zv, in1=u2h[:, a:b],
                                    op=AT.add)
            nc.vector.tensor_tensor(out=zv, in0=zv, in1=dn, op=AT.mult)
            nc.vector.tensor_scalar(out=zv, in0=zv, scalar1=b2c,
                                    scalar2=None, op0=AT.add)
            nc.sync.dma_start(out_d[a:b, :].rearrange("b p -> p b"), zv)


# ---------------------------------------------------------------------------

def kernel(**inputs):
    edge_index = np.asarray(inputs["edge_index"])
    prep = _prep(edge_index)
    C, groups, perms = prep["C"], prep["groups"], prep["perms"]

    x = np.asarray(inputs["x"], dtype=np.float32).reshape(-1)   # [N]

    flat = lambda a: np.ascontiguousarray(
        np.asarray(a, dtype=np.float32).reshape(1, -1))
    w1 = flat(inputs["W1"]); as1 = flat(inputs["att_src1"])
    ad1 = flat(inputs["att_dst1"]); b1 = flat(inputs["b1"])
    w2 = flat(inputs["W2"])
    sc2 = np.zeros((1, 8), np.float32)
    sc2[0, 0] = np.asarray(inputs["att_src2"]).reshape(-1)[0]
    sc2[0, 1] = np.asarray(inputs["att_dst2"]).reshape(-1)[0]
    sc2[0, 2] = np.asarray(inputs["b2"]).reshape(-1)[0]
    sc2[0, 4:8] = [0.0, 1.0, 2.0, 3.0]
    iota64 = np.arange(64, dtype=np.float32).reshape(1, 64)

    nc = _build(C, groups)

    in_maps = []
    for k in range(NCORES):
        xk = x[k * NPC:(k + 1) * NPC][perms[k]]
        xk = np.concatenate([xk, np.zeros(ROWS - NPC, np.float32)])
        x_own = np.ascontiguousarray(xk.reshape(NBLK, P).T)
        ia, offa = _mk_gather_inputs(prep["pos"][k], prep["valid"][k], C)
        fb = np.concatenate([
            x_own.ravel(), iota64.ravel(), w1.ravel(),
            as1.ravel(), ad1.ravel(), b1.ravel(), w2.ravel(), sc2.ravel(),
        ]).astype(np.float32)
        in_maps.append({"fblob": fb, "iblob": ia.ravel(),
                        "ublob": offa.ravel()})

    res = run_bass_kernel_spmd(nc, in_maps, core_ids=list(range(NCORES)))

    out = np.zeros((N, 1), np.float32)
    for k in range(NCORES):
        o = res.results[k]["out"]                    # [NBLK, P]
        flat_o = o.reshape(-1)[:NPC]
        out[k * NPC:(k + 1) * NPC, 0][perms[k]] = flat_o
    return out

